# revision 1
# baseline (speedup 1.0000x reference)
"""Bidirectional Mamba layer on 8 Trainium2 NeuronCores.

Sharding: data-parallel over batch (8 batches -> 8 cores). Each core runs
both directions (fwd on x, bwd on time-reversed x) for its batch.

Per-core algorithm (per direction), all in "d-major" layout [d on
partitions, time on free dim]:
  1. uzT = in_w @ x^T                (PE, bf16)
  2. causal depthwise conv + SiLU    (ACT scale-copy + 3 fused DVE STT)
  3. dblT = xp_w @ uc^T              (PE)  -> dt / B / C rows
  4. deltaT = softplus(dt_w @ dtT + dt_b)  (PE + ACT Softplus)
  5. per (d-chunk, s):  a = exp(A[d,s] * delta)   (ACT, per-partition scale)
                        b = (delta*uc) * bcast(B[s,:])  (DVE)
                        h = tensor_tensor_scan(a, b)    (DVE, fp32 state)
                        y += h * bcast(C[s,:])          (DVE)
  6. g = (uc*D + y) * silu(z); out = g^T @ out_w^T      (PE)
Host combines: out = out_f + reverse_time(out_b).
"""

import sys

sys.path.insert(0, "/opt/trn_rl_repo")

import numpy as np
import ml_dtypes

import concourse.bass as bass
import concourse.mybir as mybir
import bass_rust
from concourse import tile
from concourse.bass_utils import run_bass_kernel_spmd

BF16 = mybir.dt.bfloat16
F32 = mybir.dt.float32
AF = mybir.ActivationFunctionType
OP = mybir.AluOpType

D_MODEL = 512
D_INNER = 1024
D_STATE = 16
D_CONV = 4
DT_RANK = 32
BATCH = 8
SEQ = 1024

P = 128
NC_D = D_INNER // P  # 8 d-chunks
NC_T = SEQ // P      # 8 t-chunks
NN = SEQ // 512      # 2 psum-free chunks


def _dir_params(nc, d):
    """Declare per-direction dram parameters (host passes pre-transposed)."""
    return {
        "inwT": nc.declare_dram_parameter(f"inwT_{d}", [D_MODEL, 2 * D_INNER], BF16, isOutput=False),
        "xpwT": nc.declare_dram_parameter(f"xpwT_{d}", [D_INNER, DT_RANK + 2 * D_STATE], BF16, isOutput=False),
        "dtwT": nc.declare_dram_parameter(f"dtwT_{d}", [DT_RANK, D_INNER], BF16, isOutput=False),
        "outwT": nc.declare_dram_parameter(f"outwT_{d}", [D_INNER, D_MODEL], BF16, isOutput=False),
        "A": nc.declare_dram_parameter(f"A_{d}", [D_INNER, D_STATE], F32, isOutput=False),
        "convw": nc.declare_dram_parameter(f"convw_{d}", [D_INNER, D_CONV], F32, isOutput=False),
        "convb": nc.declare_dram_parameter(f"convb_{d}", [D_INNER, 1], F32, isOutput=False),
        "dtb": nc.declare_dram_parameter(f"dtb_{d}", [D_INNER, 1], F32, isOutput=False),
        "Dp": nc.declare_dram_parameter(f"Dp_{d}", [D_INNER, 1], F32, isOutput=False),
        "xT": nc.declare_dram_parameter(f"xT_{d}", [D_MODEL, SEQ], BF16, isOutput=False),
        "out": nc.declare_dram_parameter(f"out_{d}", [SEQ, D_MODEL], F32, isOutput=True),
        "oht": nc.declare_dram_parameter(f"oht_{d}", [2 * D_STATE, 2 * D_STATE * P], BF16, isOutput=False),
    }


def _one_direction(ctx_pools, tc, p):
    import contextlib

    nc = tc.nc

    cst = ctx_pools  # long-lived pool for this direction

    # ---- load weights ----
    inwT = [cst.tile([P, 2 * D_INNER], BF16, tag=f"inwT{k}", name=f"inwT{k}") for k in range(4)]
    for k in range(4):
        nc.sync.dma_start(inwT[k][:], p["inwT"][k * P:(k + 1) * P, :])
    xT = [cst.tile([P, SEQ], BF16, tag=f"xT{k}", name=f"xT{k}") for k in range(4)]
    for k in range(4):
        nc.sync.dma_start(xT[k][:], p["xT"][k * P:(k + 1) * P, :])
    xpwT = [cst.tile([P, 64], BF16, tag=f"xpwT{c}", name=f"xpwT{c}") for c in range(NC_D)]
    outwT = [cst.tile([P, D_MODEL], BF16, tag=f"outwT{c}", name=f"outwT{c}") for c in range(NC_D)]
    A_sb = [cst.tile([P, D_STATE], F32, tag=f"A{c}", name=f"A{c}") for c in range(NC_D)]
    convw = [cst.tile([P, D_CONV], F32, tag=f"convw{c}", name=f"convw{c}") for c in range(NC_D)]
    convb = [cst.tile([P, 1], F32, tag=f"convb{c}", name=f"convb{c}") for c in range(NC_D)]
    dtb = [cst.tile([P, 1], F32, tag=f"dtb{c}", name=f"dtb{c}") for c in range(NC_D)]
    Dp = [cst.tile([P, 1], F32, tag=f"Dp{c}", name=f"Dp{c}") for c in range(NC_D)]
    for c in range(NC_D):
        sl = slice(c * P, (c + 1) * P)
        nc.sync.dma_start(xpwT[c][:], p["xpwT"][sl, :])
        nc.sync.dma_start(outwT[c][:], p["outwT"][sl, :])
        nc.sync.dma_start(A_sb[c][:], p["A"][sl, :])
        nc.sync.dma_start(convw[c][:], p["convw"][sl, :])
        nc.sync.dma_start(convb[c][:], p["convb"][sl, :])
        nc.sync.dma_start(dtb[c][:], p["dtb"][sl, :])
        nc.sync.dma_start(Dp[c][:], p["Dp"][sl, :])
    dtwT = cst.tile([DT_RANK, D_INNER], BF16, tag="dtwT", name="dtwT")
    nc.sync.dma_start(dtwT[:], p["dtwT"][:])

    # persistent activations for this direction
    uT = [cst.tile([P, SEQ + D_CONV - 1], BF16, tag=f"uT{c}", name=f"uT{c}") for c in range(NC_D)]
    sz = [cst.tile([P, SEQ], BF16, tag=f"sz{c}", name=f"sz{c}") for c in range(NC_D)]
    ucT = [cst.tile([P, SEQ], BF16, tag=f"ucT{c}", name=f"ucT{c}") for c in range(NC_D)]
    delta = [cst.tile([P, SEQ], BF16, tag=f"delta{c}", name=f"delta{c}") for c in range(NC_D)]
    w_bf = [cst.tile([P, SEQ], BF16, tag=f"w{c}", name=f"w{c}") for c in range(NC_D)]
    y_sb = [cst.tile([P, SEQ], BF16, tag=f"y{c}", name=f"y{c}") for c in range(NC_D)]
    dt_bf = cst.tile([DT_RANK, SEQ], BF16, tag="dt_bf", name="dt_bf")
    bc_bf = cst.tile([2 * D_STATE, SEQ], BF16, tag="bc_bf", name="bc_bf")

    for c in range(NC_D):
        nc.vector.memset(uT[c][:, 0:D_CONV - 1], 0.0)

    with contextlib.ExitStack() as phase:
        ps1 = phase.enter_context(tc.tile_pool(name="ps1", bufs=4, space="PSUM"))
        # ---- GEMM1: uzT[m*128:(m+1)*128, :] ----
        for m in range(2 * NC_D):
            for n in range(NN):
                pt = ps1.tile([P, 512], F32, tag="g1", name="g1")
                for k in range(4):
                    nc.tensor.matmul(
                        pt[:],
                        inwT[k][:, m * P:(m + 1) * P],
                        xT[k][:, n * 512:(n + 1) * 512],
                        start=(k == 0),
                        stop=(k == 3),
                    )
                if m < NC_D:
                    nc.scalar.copy(
                        uT[m][:, D_CONV - 1 + n * 512: D_CONV - 1 + (n + 1) * 512],
                        pt[:],
                    )
                else:
                    nc.scalar.activation(
                        sz[m - NC_D][:, n * 512:(n + 1) * 512], pt[:], AF.Silu
                    )

        # ---- conv + SiLU ----
        t_pool = phase.enter_context(tc.tile_pool(name="conv_t", bufs=2))
        for c in range(NC_D):
            taps = []
            for k in range(D_CONV):
                tk = t_pool.tile([P, SEQ], BF16, tag="tk", name="tk", bufs=5)
                nc.scalar.activation(
                    tk[:], uT[c][:, k:k + SEQ], AF.Copy, scale=convw[c][:, k:k + 1]
                )
                taps.append(tk)
            s01 = t_pool.tile([P, SEQ], BF16, tag="s01", name="s01")
            nc.vector.tensor_add(s01[:], taps[0][:], taps[1][:])
            s23 = t_pool.tile([P, SEQ], BF16, tag="s23", name="s23")
            nc.vector.tensor_add(s23[:], taps[2][:], taps[3][:])
            s03 = t_pool.tile([P, SEQ], BF16, tag="s03", name="s03")
            nc.vector.tensor_add(s03[:], s01[:], s23[:])
            nc.scalar.activation(
                ucT[c][:], s03[:], AF.Silu, bias=convb[c][:, 0:1]
            )

    with contextlib.ExitStack() as phase:
        ps2 = phase.enter_context(tc.tile_pool(name="ps2", bufs=4, space="PSUM"))
        # ---- GEMM2: dblT [64, SEQ] ----
        for n in range(NN):
            pt = ps2.tile([64, 512], F32, tag="g2", name="g2")
            for c in range(NC_D):
                nc.tensor.matmul(
                    pt[:], xpwT[c][:], ucT[c][:, n * 512:(n + 1) * 512],
                    start=(c == 0), stop=(c == NC_D - 1),
                )
            nc.vector.tensor_copy(dt_bf[:, n * 512:(n + 1) * 512], pt[0:DT_RANK, :])
            nc.vector.tensor_copy(
                bc_bf[:, n * 512:(n + 1) * 512], pt[DT_RANK:64, :]
            )

        # ---- GEMM3: deltaT = softplus(dt_w @ dtT + dt_b) ----
        # softplus(x) = relu(x) + ln(1 + exp(-|x|))  (Softplus has no ACT table set)
        t_pool2 = phase.enter_context(tc.tile_pool(name="sp_t", bufs=3))
        for m in range(NC_D):
            for n in range(NN):
                pt = ps2.tile([P, 512], F32, tag="g3", name="g3")
                nc.tensor.matmul(
                    pt[:], dtwT[:, m * P:(m + 1) * P],
                    dt_bf[:, n * 512:(n + 1) * 512],
                    start=True, stop=True,
                )
                sl = slice(n * 512, (n + 1) * 512)
                ab = t_pool2.tile([P, 512], F32, tag="sp_ab", name="sp_ab")
                nc.scalar.activation(ab[:], pt[:], AF.Abs, bias=dtb[m][:, 0:1])
                en = t_pool2.tile([P, 512], F32, tag="sp_en", name="sp_en")
                nc.scalar.activation(en[:], ab[:], AF.Exp, scale=-1.0)
                l1 = t_pool2.tile([P, 512], F32, tag="sp_l1", name="sp_l1")
                nc.scalar.activation(l1[:], en[:], AF.Ln, bias=1.0)
                rl = t_pool2.tile([P, 512], F32, tag="sp_rl", name="sp_rl")
                nc.scalar.activation(rl[:], pt[:], AF.Relu, bias=dtb[m][:, 0:1])
                nc.vector.tensor_add(delta[m][:, sl], rl[:], l1[:])

        # ---- w = delta * uc ----
        for c in range(NC_D):
            nc.vector.tensor_mul(w_bf[c][:], delta[c][:], ucT[c][:])

    # ---- scan phase ----
    oht = cst.tile([2 * D_STATE, 2 * D_STATE * P], BF16, tag="oht", name="oht")
    nc.sync.dma_start(oht[:], p["oht"][:])
    with contextlib.ExitStack() as phase:
        bcp_pool = phase.enter_context(tc.tile_pool(name="bcp", bufs=4, space="PSUM"))
        bc_pool = phase.enter_context(tc.tile_pool(name="bc", bufs=3))
        ab_pool = phase.enter_context(tc.tile_pool(name="ab", bufs=4))
        h_pool = phase.enter_context(tc.tile_pool(name="h", bufs=3))
        for s in range(D_STATE):
            Bbc = bc_pool.tile([P, SEQ], BF16, tag="Bbc", name="Bbc")
            Cbc = bc_pool.tile([P, SEQ], BF16, tag="Cbc", name="Cbc")
            for src_row, dst in ((s, Bbc), (D_STATE + s, Cbc)):
                ps = bcp_pool.tile([P, SEQ], F32, tag="bcps", name="bcps")
                for n in range(NN):
                    nc.tensor.matmul(
                        ps[:, n * 512:(n + 1) * 512],
                        oht[:, src_row * P:(src_row + 1) * P],
                        bc_bf[:, n * 512:(n + 1) * 512],
                        start=True,
                        stop=True,
                    )
                nc.scalar.copy(dst[:], ps[:])
            for c in range(NC_D):
                a_t = ab_pool.tile([P, SEQ], BF16, tag="a", name="a")
                nc.scalar.activation(
                    a_t[:], delta[c][:], AF.Exp, scale=A_sb[c][:, s:s + 1]
                )
                b_t = ab_pool.tile([P, SEQ], BF16, tag="b", name="b")
                nc.vector.tensor_mul(b_t[:], w_bf[c][:], Bbc[:])
                h_t = h_pool.tile([P, SEQ], BF16, tag="h", name="h")
                nc.vector.tensor_tensor_scan(
                    h_t[:], a_t[:], b_t[:], 0.0, op0=OP.mult, op1=OP.add
                )
                if s == 0:
                    nc.vector.tensor_mul(y_sb[c][:], h_t[:], Cbc[:])
                else:
                    t_t = h_pool.tile([P, SEQ], BF16, tag="yt", name="yt")
                    nc.vector.tensor_mul(t_t[:], h_t[:], Cbc[:])
                    nc.vector.tensor_add(y_sb[c][:], y_sb[c][:], t_t[:])

    # ---- gate: y = (uc*D + y) * silu(z) ----
    for c in range(NC_D):
        ucd = cst.tile([P, SEQ], BF16, tag=f"ucd{c}", name=f"ucd{c}")
        nc.scalar.activation(ucd[:], ucT[c][:], AF.Copy, scale=Dp[c][:, 0:1])
        nc.vector.tensor_add(y_sb[c][:], y_sb[c][:], ucd[:])
        nc.vector.tensor_mul(y_sb[c][:], y_sb[c][:], sz[c][:])

    # ---- GEMM4: out[m*128:(m+1)*128, :] = g^T @ out_w^T ----
    with contextlib.ExitStack() as phase:
        ps4 = phase.enter_context(tc.tile_pool(name="ps4", bufs=3, space="PSUM"))
        o_pool = phase.enter_context(tc.tile_pool(name="o", bufs=3))
        for m in range(NC_T):
            pt = ps4.tile([P, D_MODEL], F32, tag="g4", name="g4")
            for c in range(NC_D):
                nc.tensor.matmul(
                    pt[:], y_sb[c][:, m * P:(m + 1) * P], outwT[c][:],
                    start=(c == 0), stop=(c == NC_D - 1),
                )
            ot = o_pool.tile([P, D_MODEL], F32, tag="ot", name="ot")
            nc.vector.tensor_copy(ot[:], pt[:])
            nc.sync.dma_start(p["out"][m * P:(m + 1) * P, :], ot[:])


def _split_excess_waits(nc):
    """walrus in this toolchain accepts at most one sync-wait per
    instruction (two for EventSemaphore); hoist the excess onto injected
    same-engine NoOps placed directly before the instruction."""
    for f in nc.m.functions:
        for bb in f.blocks:
            new_insts = []
            for inst in bb.instructions:
                si = inst.sync_info
                cap = 2 if isinstance(inst, mybir.InstEventSemaphore) else 1
                if si is not None and len(si.on_wait) > cap:
                    waits = list(si.on_wait)
                    for i, w in enumerate(waits[:-cap]):
                        nop = mybir.InstNoOp(
                            name=f"{inst.name}-wsplit{i}", ins=[], outs=[]
                        )
                        nop.engine = inst.engine
                        nop.sync_info = bass_rust.SyncInfo(on_wait=[w], on_update=[])
                        new_insts.append(nop)
                    inst.sync_info = bass_rust.SyncInfo(
                        on_wait=waits[-cap:], on_update=list(si.on_update)
                    )
                new_insts.append(inst)
            try:
                bb.instructions = new_insts
            except Exception:
                bb.instructions.clear()
                bb.instructions.extend(new_insts)


def build_bass():
    nc = bass.Bass()
    params = {d: _dir_params(nc, d) for d in ("f", "b")}
    with tile.TileContext(nc) as tc:
        for d in ("f", "b"):
            with tc.tile_pool(name=f"cst_{d}", bufs=1) as cst:
                _one_direction(cst, tc, params[d])
    _split_excess_waits(nc)
    return nc


def _prep_dir(w):
    """Host-side prep of one direction's weights -> dram param arrays."""
    bf = ml_dtypes.bfloat16
    in_w, conv_w, conv_b, xp_w, dt_w, dt_b, A_log, Dp, out_w = w
    return {
        "inwT": np.ascontiguousarray(in_w.T).astype(bf),
        "xpwT": np.ascontiguousarray(xp_w.T).astype(bf),
        "dtwT": np.ascontiguousarray(dt_w.T).astype(bf),
        "outwT": np.ascontiguousarray(out_w.T).astype(bf),
        "A": np.ascontiguousarray(-np.exp(A_log.astype(np.float64))).astype(np.float32),
        "convw": np.ascontiguousarray(conv_w).astype(np.float32),
        "convb": np.ascontiguousarray(conv_b).reshape(D_INNER, 1).astype(np.float32),
        "dtb": np.ascontiguousarray(dt_b).reshape(D_INNER, 1).astype(np.float32),
        "Dp": np.ascontiguousarray(Dp).reshape(D_INNER, 1).astype(np.float32),
        "oht": np.kron(np.eye(2 * D_STATE, dtype=np.float32), np.ones((1, P), np.float32)).astype(bf),
    }


_CACHED = {}


def kernel(
    x,
    in_w_f, conv_w_f, conv_b_f, xp_w_f, dt_w_f, dt_b_f, A_log_f, D_f, out_w_f,
    in_w_b, conv_w_b, conv_b_b, xp_w_b, dt_w_b, dt_b_b, A_log_b, D_b, out_w_b,
):
    bf = ml_dtypes.bfloat16
    x = np.asarray(x, dtype=np.float32)

    if "nc" not in _CACHED:
        _CACHED["nc"] = build_bass()
    nc = _CACHED["nc"]

    wf = _prep_dir((in_w_f, conv_w_f, conv_b_f, xp_w_f, dt_w_f, dt_b_f,
                    A_log_f, D_f, out_w_f))
    wb = _prep_dir((in_w_b, conv_w_b, conv_b_b, xp_w_b, dt_w_b, dt_b_b,
                    A_log_b, D_b, out_w_b))

    in_maps = []
    for b in range(BATCH):
        m = {}
        for d, wd in (("f", wf), ("b", wb)):
            for k, v in wd.items():
                m[f"{k}_{d}"] = v
        m["xT_f"] = np.ascontiguousarray(x[b].T).astype(bf)
        m["xT_b"] = np.ascontiguousarray(x[b][::-1].T).astype(bf)
        in_maps.append(m)

    res = run_bass_kernel_spmd(nc, in_maps, core_ids=list(range(BATCH)))
    out = np.empty((BATCH, SEQ, D_MODEL), np.float32)
    for b in range(BATCH):
        rb = res.results[b]
        out[b] = rb["out_f"] + rb["out_b"][::-1]
    return out



# revision 13
# speedup vs baseline: 1.4796x; 1.4796x over previous
"""Bidirectional Mamba layer on 8 Trainium2 NeuronCores.

Sharding: data-parallel over batch (8 batches -> 8 cores). Each core runs
both directions (fwd on x, bwd via reversed-stride reads of the same x).

Per-core algorithm per direction, d-major layout [d on partitions, t free]:
  1. uzT = in_w @ x^T                   (PE; bwd reads x with stride -1)
  2. causal depthwise conv via PE diag(conv_w[:,k]) matmuls, SiLU on ACT
  3. dblT = xp_w @ uc^T                 (PE)  -> dt / B / C rows
  4. per chunk c (lazy): delta = ln(1+exp(dt_w@dtT + dt_b)) (2 ACT ops,
     shares the natural_log_exp table with the scan's exp), w = delta*uc
  5. B/C rows broadcast to 128 partitions via PE one-hot matmuls, copied
     to wide per-state-pair SBUF tiles (Pool)
  6. per (c, s): a = exp(A[:,s]*delta) (ACT); b = w*Bbc (DVE/Pool);
     h = tensor_tensor_scan(a, b) (DVE); hc = h*Cbc (DVE/Pool);
     psum_y += I @ hc (PE, fp32 accumulate; initialized with diag(D)@uc)
  7. yg = psum_y * silu(z)   (DVE, PSUM operand)
  8. out = yg^T @ out_w^T    (PE)
Host combines: out = out_f + reverse_time(out_b).
"""

import sys

sys.path.insert(0, "/opt/trn_rl_repo")

import numpy as np
import ml_dtypes

import concourse.bass as bass
import concourse.mybir as mybir
import bass_rust
from concourse import tile
from concourse.bass_utils import run_bass_kernel_spmd

BF16 = mybir.dt.bfloat16
F32 = mybir.dt.float32
AF = mybir.ActivationFunctionType
OP = mybir.AluOpType

D_MODEL = 512
D_INNER = 1024
D_STATE = 16
D_CONV = 4
DT_RANK = 32
BATCH = 8
SEQ = 1024

P = 128
NC_D = D_INNER // P   # 8 d-chunks
NC_T = SEQ // P       # 8 t-chunks
NN = SEQ // 512       # 2 psum-free blocks
NSP = D_STATE // 2    # 8 state pairs

# Engine split for the per-(c,sp) muls, tuned against the tile cost model:
# b2-mul always on Pool; hc2-mul on Pool every POOL_HC_MOD-th pair.
POOL_HC_MOD = 4


def _dir_params(nc, d):
    return {
        "inwT": nc.declare_dram_parameter(f"inwT_{d}", [D_MODEL, 2 * D_INNER], BF16, isOutput=False),
        "xpwT": nc.declare_dram_parameter(f"xpwT_{d}", [D_INNER, DT_RANK + 2 * D_STATE], BF16, isOutput=False),
        "dtwT": nc.declare_dram_parameter(f"dtwT_{d}", [DT_RANK, D_INNER], BF16, isOutput=False),
        "outwT": nc.declare_dram_parameter(f"outwT_{d}", [D_INNER, D_MODEL], BF16, isOutput=False),
        "A": nc.declare_dram_parameter(f"A_{d}", [D_INNER, D_STATE], F32, isOutput=False),
        "convdiag": nc.declare_dram_parameter(f"convdiag_{d}", [NC_D * D_CONV * P, P], BF16, isOutput=False),
        "ddiag": nc.declare_dram_parameter(f"ddiag_{d}", [NC_D * P, P], BF16, isOutput=False),
        "convb": nc.declare_dram_parameter(f"convb_{d}", [D_INNER, 1], F32, isOutput=False),
        "dtb": nc.declare_dram_parameter(f"dtb_{d}", [D_INNER, 1], F32, isOutput=False),
        "out": nc.declare_dram_parameter(f"out_{d}", [SEQ, D_MODEL], F32, isOutput=True),
    }


class Shared:
    pass


def _build_shared(nc, tc, pools):
    """Pools + direction-independent tiles (identity, oht, x)."""
    sh = Shared()
    sh.xT_d = nc.declare_dram_parameter("xT", [D_MODEL, SEQ], BF16, isOutput=False)
    sh.oht_d = nc.declare_dram_parameter("oht", [2 * D_STATE, 2 * D_STATE * P], BF16, isOutput=False)
    sh.ident_d = nc.declare_dram_parameter("ident", [P, P], BF16, isOutput=False)

    cst = pools["cst"]
    sh.xT = [cst.tile([P, SEQ], BF16, tag=f"xT{k}", name=f"xT{k}") for k in range(4)]
    for k in range(4):
        nc.sync.dma_start(sh.xT[k][:], sh.xT_d[k * P:(k + 1) * P, :])
    sh.ident = cst.tile([P, P], BF16, tag="ident", name="ident")
    nc.sync.dma_start(sh.ident[:], sh.ident_d[:])
    return sh


def _one_direction(nc, tc, pools, sh, p, rev):
    """Emit one direction. rev=True reads x time-reversed."""
    cst = pools["cst"]
    wpool = pools["w"]          # rotating weight tiles (per-dir tags shared)
    ps_small = pools["ps_small"]
    ps_y = pools["ps_y"]

    # ---- weights (rotating by shared tags; dir b reuses dir f's buffers) ----
    inwT = [wpool.tile([P, 2 * D_INNER], BF16, tag=f"inwT{k}", name=f"inwT{k}") for k in range(4)]
    for k in range(4):
        nc.sync.dma_start(inwT[k][:], p["inwT"][k * P:(k + 1) * P, :])
    xpwT = [wpool.tile([P, 64], BF16, tag=f"xpwT{c}", name=f"xpwT{c}") for c in range(NC_D)]
    A_sb = [wpool.tile([P, D_STATE], F32, tag=f"A{c}", name=f"A{c}") for c in range(NC_D)]
    convb = [wpool.tile([P, 1], F32, tag=f"convb{c}", name=f"convb{c}") for c in range(NC_D)]
    dtb = [wpool.tile([P, 1], F32, tag=f"dtb{c}", name=f"dtb{c}") for c in range(NC_D)]
    for c in range(NC_D):
        sl = slice(c * P, (c + 1) * P)
        nc.sync.dma_start(xpwT[c][:], p["xpwT"][sl, :])
        nc.sync.dma_start(A_sb[c][:], p["A"][sl, :])
        nc.sync.dma_start(convb[c][:], p["convb"][sl, :])
        nc.sync.dma_start(dtb[c][:], p["dtb"][sl, :])
    dtwT = wpool.tile([DT_RANK, D_INNER], BF16, tag="dtwT", name="dtwT")
    nc.sync.dma_start(dtwT[:], p["dtwT"][:])

    uc = [pools["uc"].tile([P, SEQ], BF16, tag="uc", name=f"uc{c}") for c in range(NC_D)]
    sz = [pools["sz"].tile([P, SEQ], BF16, tag="sz", name=f"sz{c}") for c in range(NC_D)]

    def xs(n):
        """x k-tile slice for GEMM1 block n, reversed for the bwd dir."""
        if not rev:
            return [sh.xT[k][:, n * 512:(n + 1) * 512] for k in range(4)]
        # reversed block n reads original columns SEQ-1-n*512 .. SEQ-(n+1)*512
        start = SEQ - 1 - n * 512
        stop = SEQ - 1 - (n + 1) * 512
        if stop < 0:
            return [sh.xT[k][:, start::-1] for k in range(4)]
        return [sh.xT[k][:, start:stop:-1] for k in range(4)]

    # ---- GEMM1-u + conv fused per chunk ----
    for c in range(NC_D):
        uT = pools["uT"].tile([P, SEQ + D_CONV - 1], BF16, tag="uT", name=f"uT{c}")
        nc.vector.memset(uT[:, 0:D_CONV - 1], 0.0)
        for n in range(NN):
            pt = ps_small.tile([P, 512], F32, tag="g1", name="g1u")
            for k in range(4):
                nc.tensor.matmul(
                    pt[:], inwT[k][:, c * P:(c + 1) * P], xs(n)[k],
                    start=(k == 0), stop=(k == 3),
                )
            nc.scalar.copy(uT[:, D_CONV - 1 + n * 512:D_CONV - 1 + (n + 1) * 512], pt[:])
        # conv: uc_psum = sum_k diag(convw_k) @ u(t+k-3)
        cd = [pools["cdiag"].tile([P, P], BF16, tag="cdiag", name=f"cd{c}_{k}") for k in range(D_CONV)]
        for k in range(D_CONV):
            nc.sync.dma_start(cd[k][:], p["convdiag"][(c * D_CONV + k) * P:(c * D_CONV + k + 1) * P, :])
        for n in range(NN):
            pt = ps_small.tile([P, 512], F32, tag="g1", name="g1c")
            for k in range(D_CONV):
                nc.tensor.matmul(
                    pt[:], cd[k][:], uT[:, k + n * 512:k + n * 512 + 512],
                    start=(k == 0), stop=(k == D_CONV - 1),
                )
            nc.scalar.activation(
                uc[c][:, n * 512:(n + 1) * 512], pt[:], AF.Silu, bias=convb[c][:, 0:1]
            )

    # ---- GEMM1-z + silu ----
    for c in range(NC_D):
        for n in range(NN):
            pt = ps_small.tile([P, 512], F32, tag="g1", name="g1z")
            for k in range(4):
                nc.tensor.matmul(
                    pt[:], inwT[k][:, D_INNER + c * P:D_INNER + (c + 1) * P], xs(n)[k],
                    start=(k == 0), stop=(k == 3),
                )
            nc.scalar.activation(sz[c][:, n * 512:(n + 1) * 512], pt[:], AF.Silu)

    # ---- GEMM2: dbl = xp_w @ uc ----
    dt_bf = pools["dtbf"].tile([DT_RANK, SEQ], BF16, tag="dt_bf", name="dt_bf")
    bc_bf = pools["bcbf"].tile([2 * D_STATE, SEQ], BF16, tag="bc_bf", name="bc_bf")
    for n in range(NN):
        pt = ps_small.tile([64, 512], F32, tag="g1", name="g2")
        for c in range(NC_D):
            nc.tensor.matmul(
                pt[:], xpwT[c][:], uc[c][:, n * 512:(n + 1) * 512],
                start=(c == 0), stop=(c == NC_D - 1),
            )
        nc.vector.tensor_copy(dt_bf[:, n * 512:(n + 1) * 512], pt[0:DT_RANK, :])
        nc.vector.tensor_copy(bc_bf[:, n * 512:(n + 1) * 512], pt[DT_RANK:64, :])

    # ---- broadcast B/C rows to wide per-pair tiles [P, 2048] ----
    Bbc = [pools["bc"].tile([P, 2048], BF16, tag=f"Bbc{sp}", name=f"Bbc{sp}") for sp in range(NSP)]
    Cbc = [pools["bc"].tile([P, 2048], BF16, tag=f"Cbc{sp}", name=f"Cbc{sp}") for sp in range(NSP)]
    for sp in range(NSP):
        for half, dst in ((0, Bbc[sp]), (1, Cbc[sp])):
            for j in range(2):
                row = half * D_STATE + sp * 2 + j
                ohs = pools["oht"].tile([2 * D_STATE, P], BF16, tag="oht", name=f"oh{row}")
                nc.sync.dma_start(ohs[:], sh.oht_d[:, row * P:(row + 1) * P])
                for n in range(NN):
                    pt = ps_small.tile([P, 512], F32, tag="g1", name="bcb")
                    nc.tensor.matmul(
                        pt[:], ohs[:],
                        bc_bf[:, n * 512:(n + 1) * 512],
                        start=True, stop=True,
                    )
                    nc.scalar.copy(
                        dst[:, j * 1024 + n * 512:j * 1024 + (n + 1) * 512], pt[:]
                    )

    # ---- scan phase, chunk-outer ----
    yg = [pools["yg"].tile([P, SEQ], BF16, tag="yg", name=f"yg{c}") for c in range(NC_D)]
    for c in range(NC_D):
        # lazy delta_c = softplus(dt_w @ dt + dtb), w_c = delta_c * uc_c
        pt = ps_small.tile([P, 512], F32, tag="g1", name="g3a")
        pt2 = ps_small.tile([P, 512], F32, tag="g1", name="g3b")
        nc.tensor.matmul(pt[:], dtwT[:, c * P:(c + 1) * P], dt_bf[:, 0:512],
                         start=True, stop=True)
        nc.tensor.matmul(pt2[:], dtwT[:, c * P:(c + 1) * P], dt_bf[:, 512:1024],
                         start=True, stop=True)
        delta = pools["delta"].tile([P, SEQ], BF16, tag="delta", name=f"delta{c}")
        nc.scalar.activation(delta[:, 0:512], pt[:], AF.Exp, bias=dtb[c][:, 0:1])
        nc.scalar.activation(delta[:, 512:1024], pt2[:], AF.Exp, bias=dtb[c][:, 0:1])
        nc.scalar.activation(delta[:], delta[:], AF.Ln, bias=1.0)
        w_t = pools["wt"].tile([P, SEQ], BF16, tag="wt", name=f"w{c}")
        nc.vector.tensor_mul(w_t[:], delta[:], uc[c][:])
        w_b = w_t[:].unsqueeze(1).broadcast_to((P, 2, 1024))

        # D * uc seeds the PSUM accumulator
        dd = pools["ddiag"].tile([P, P], BF16, tag="ddiag", name=f"dd{c}")
        nc.sync.dma_start(dd[:], p["ddiag"][c * P:(c + 1) * P, :])
        py = ps_y.tile([P, SEQ], F32, tag="py", name=f"py{c}")
        for n in range(NN):
            nc.tensor.matmul(py[:, n * 512:(n + 1) * 512], dd[:],
                             uc[c][:, n * 512:(n + 1) * 512], start=True, stop=False)

        for sp in range(NSP):
            a2 = pools["a2"].tile([P, 2048], BF16, tag="a2", name="a2")
            for j in range(2):
                nc.scalar.activation(
                    a2[:, j * 1024:(j + 1) * 1024], delta[:], AF.Exp,
                    scale=A_sb[c][:, sp * 2 + j:sp * 2 + j + 1],
                )
            b2 = pools["b2"].tile([P, 2048], BF16, tag="b2", name="b2")
            nc.gpsimd.tensor_tensor(
                b2[:].rearrange("p (s n) -> p s n", s=2), w_b,
                Bbc[sp][:].rearrange("p (s n) -> p s n", s=2), OP.mult,
            )
            h2 = pools["h2"].tile([P, 2048], BF16, tag="h2", name="h2")
            for j in range(2):
                nc.vector.tensor_tensor_scan(
                    h2[:, j * 1024:(j + 1) * 1024],
                    a2[:, j * 1024:(j + 1) * 1024],
                    b2[:, j * 1024:(j + 1) * 1024],
                    0.0, op0=OP.mult, op1=OP.add,
                )
            hc2 = pools["hc2"].tile([P, 2048], BF16, tag="hc2", name="hc2")
            hmul = nc.gpsimd if (c * NSP + sp) % POOL_HC_MOD == 0 else nc.vector
            hmul.tensor_mul(hc2[:], h2[:], Cbc[sp][:])
            last = sp == NSP - 1
            for j in range(2):
                for n in range(NN):
                    nc.tensor.matmul(
                        py[:, n * 512:(n + 1) * 512], sh.ident[:],
                        hc2[:, j * 1024 + n * 512:j * 1024 + (n + 1) * 512],
                        start=False, stop=(last and j == 1),
                    )
        # gate
        nc.vector.tensor_mul(yg[c][:], py[:], sz[c][:])

    # ---- GEMM4 ----
    outwT = [wpool.tile([P, D_MODEL], BF16, tag=f"outwT{c}", name=f"outwT{c}") for c in range(NC_D)]
    for c in range(NC_D):
        nc.sync.dma_start(outwT[c][:], p["outwT"][c * P:(c + 1) * P, :])
    for m in range(NC_T):
        pt = ps_small.tile([P, D_MODEL], F32, tag="g1", name="g4")
        for c in range(NC_D):
            nc.tensor.matmul(
                pt[:], yg[c][:, m * P:(m + 1) * P], outwT[c][:],
                start=(c == 0), stop=(c == NC_D - 1),
            )
        ot = pools["g4o"].tile([P, D_MODEL], F32, tag="g4o", name="ot")
        nc.scalar.copy(ot[:], pt[:])
        nc.sync.dma_start(p["out"][m * P:(m + 1) * P, :], ot[:])


def _split_excess_waits(nc):
    """walrus accepts at most one sync-wait per instruction (two for
    EventSemaphore); hoist the excess onto injected same-engine NoOps."""
    for f in nc.m.functions:
        for bb in f.blocks:
            new_insts = []
            for inst in bb.instructions:
                si = inst.sync_info
                cap = 2 if isinstance(inst, mybir.InstEventSemaphore) else 1
                if si is not None and len(si.on_wait) > cap:
                    waits = list(si.on_wait)
                    for i, w in enumerate(waits[:-cap]):
                        nop = mybir.InstNoOp(
                            name=f"{inst.name}-wsplit{i}", ins=[], outs=[]
                        )
                        nop.engine = inst.engine
                        nop.sync_info = bass_rust.SyncInfo(on_wait=[w], on_update=[])
                        new_insts.append(nop)
                    inst.sync_info = bass_rust.SyncInfo(
                        on_wait=waits[-cap:], on_update=list(si.on_update)
                    )
                new_insts.append(inst)
            try:
                bb.instructions = new_insts
            except Exception:
                bb.instructions.clear()
                bb.instructions.extend(new_insts)


def build_bass():
    nc = bass.Bass()
    params = {d: _dir_params(nc, d) for d in ("f", "b")}
    with tile.TileContext(nc) as tc:
        import contextlib
        with contextlib.ExitStack() as st:
            pools = {
                "cst": st.enter_context(tc.tile_pool(name="cst", bufs=1)),
                "w": st.enter_context(tc.tile_pool(name="w", bufs=1)),
                "uT": st.enter_context(tc.tile_pool(name="uT", bufs=2)),
                "uc": st.enter_context(tc.tile_pool(name="uc", bufs=8)),
                "sz": st.enter_context(tc.tile_pool(name="sz", bufs=8)),
                "yg": st.enter_context(tc.tile_pool(name="yg", bufs=8)),
                "delta": st.enter_context(tc.tile_pool(name="delta", bufs=3)),
                "wt": st.enter_context(tc.tile_pool(name="wt", bufs=2)),
                "dtbf": st.enter_context(tc.tile_pool(name="dtbf", bufs=2)),
                "bcbf": st.enter_context(tc.tile_pool(name="bcbf", bufs=2)),
                "bc": st.enter_context(tc.tile_pool(name="bc", bufs=1)),
                "cdiag": st.enter_context(tc.tile_pool(name="cdiag", bufs=4)),
                "ddiag": st.enter_context(tc.tile_pool(name="ddiag", bufs=2)),
                "oht": st.enter_context(tc.tile_pool(name="oht", bufs=4)),
                "a2": st.enter_context(tc.tile_pool(name="a2", bufs=2)),
                "b2": st.enter_context(tc.tile_pool(name="b2", bufs=2)),
                "h2": st.enter_context(tc.tile_pool(name="h2", bufs=2)),
                "hc2": st.enter_context(tc.tile_pool(name="hc2", bufs=2)),
                "g4o": st.enter_context(tc.tile_pool(name="g4o", bufs=1)),
                "ps_small": st.enter_context(tc.tile_pool(name="ps_small", bufs=4, space="PSUM")),
                "ps_y": st.enter_context(tc.tile_pool(name="ps_y", bufs=2, space="PSUM")),
            }
            sh = _build_shared(nc, tc, pools)
            _one_direction(nc, tc, pools, sh, params["f"], rev=False)
            _one_direction(nc, tc, pools, sh, params["b"], rev=True)
    _split_excess_waits(nc)
    return nc


def _prep_dir(w):
    bf = ml_dtypes.bfloat16
    in_w, conv_w, conv_b, xp_w, dt_w, dt_b, A_log, Dp, out_w = w
    conv_w = np.asarray(conv_w, np.float32)
    convdiag = np.zeros((NC_D * D_CONV * P, P), np.float32)
    for c in range(NC_D):
        for k in range(D_CONV):
            blk = (c * D_CONV + k) * P
            convdiag[blk:blk + P, :] = np.diag(conv_w[c * P:(c + 1) * P, k])
    Dp = np.asarray(Dp, np.float32)
    ddiag = np.zeros((NC_D * P, P), np.float32)
    for c in range(NC_D):
        ddiag[c * P:(c + 1) * P, :] = np.diag(Dp[c * P:(c + 1) * P])
    return {
        "inwT": np.ascontiguousarray(np.asarray(in_w).T).astype(bf),
        "xpwT": np.ascontiguousarray(np.asarray(xp_w).T).astype(bf),
        "dtwT": np.ascontiguousarray(np.asarray(dt_w).T).astype(bf),
        "outwT": np.ascontiguousarray(np.asarray(out_w).T).astype(bf),
        "A": np.ascontiguousarray(-np.exp(np.asarray(A_log, np.float64))).astype(np.float32),
        "convdiag": convdiag.astype(bf),
        "ddiag": ddiag.astype(bf),
        "convb": np.ascontiguousarray(np.asarray(conv_b)).reshape(D_INNER, 1).astype(np.float32),
        "dtb": np.ascontiguousarray(np.asarray(dt_b)).reshape(D_INNER, 1).astype(np.float32),
    }


_CACHED = {}


def kernel(
    x,
    in_w_f, conv_w_f, conv_b_f, xp_w_f, dt_w_f, dt_b_f, A_log_f, D_f, out_w_f,
    in_w_b, conv_w_b, conv_b_b, xp_w_b, dt_w_b, dt_b_b, A_log_b, D_b, out_w_b,
):
    bf = ml_dtypes.bfloat16
    x = np.asarray(x, dtype=np.float32)

    if "nc" not in _CACHED:
        _CACHED["nc"] = build_bass()
    nc = _CACHED["nc"]

    wf = _prep_dir((in_w_f, conv_w_f, conv_b_f, xp_w_f, dt_w_f, dt_b_f,
                    A_log_f, D_f, out_w_f))
    wb = _prep_dir((in_w_b, conv_w_b, conv_b_b, xp_w_b, dt_w_b, dt_b_b,
                    A_log_b, D_b, out_w_b))
    oht = np.kron(np.eye(2 * D_STATE, dtype=np.float32),
                  np.ones((1, P), np.float32)).astype(bf)
    ident = np.eye(P, dtype=np.float32).astype(bf)

    in_maps = []
    for b in range(BATCH):
        m = {"oht": oht, "ident": ident}
        for d, wd in (("f", wf), ("b", wb)):
            for k, v in wd.items():
                m[f"{k}_{d}"] = v
        m["xT"] = np.ascontiguousarray(x[b].T).astype(bf)
        in_maps.append(m)

    res = run_bass_kernel_spmd(nc, in_maps, core_ids=list(range(BATCH)))
    out = np.empty((BATCH, SEQ, D_MODEL), np.float32)
    for b in range(BATCH):
        rb = res.results[b]
        out[b] = rb["out_f"] + rb["out_b"][::-1]
    return out


# revision 19
# speedup vs baseline: 1.6842x; 1.1383x over previous
"""Bidirectional Mamba layer on 8 Trainium2 NeuronCores.

Sharding: data-parallel over batch (8 batches -> 8 cores). Each core runs
both directions (fwd on x, bwd via reversed-stride reads of the same x).

Per-core algorithm per direction, d-major layout [d on partitions, t free]:
  1. uzT = in_w @ x^T                   (PE; bwd reads x with stride -1)
  2. causal depthwise conv via PE diag(conv_w[:,k]) matmuls, SiLU on ACT
  3. dblT = xp_w @ uc^T                 (PE)  -> dt / B / C rows
  4. per chunk c (lazy): delta = ln(1+exp(dt_w@dtT + dt_b)) (2 ACT ops,
     shares the natural_log_exp table with the scan's exp), w = delta*uc
  5. B/C rows broadcast to 128 partitions via PE one-hot matmuls, copied
     to wide per-state-pair SBUF tiles (Pool)
  6. per (c, s): a = exp(A[:,s]*delta) (ACT); b = w*Bbc (DVE/Pool);
     h = tensor_tensor_scan(a, b) (DVE); hc = h*Cbc (DVE/Pool);
     psum_y += I @ hc (PE, fp32 accumulate; initialized with diag(D)@uc)
  7. yg = psum_y * silu(z)   (DVE, PSUM operand)
  8. out = yg^T @ out_w^T    (PE)
Host combines: out = out_f + reverse_time(out_b).
"""

import sys

sys.path.insert(0, "/opt/trn_rl_repo")

import numpy as np
import ml_dtypes

import concourse.bass as bass
import concourse.mybir as mybir
import bass_rust
from concourse import tile
from concourse.bass_utils import run_bass_kernel_spmd

BF16 = mybir.dt.bfloat16
F32 = mybir.dt.float32
AF = mybir.ActivationFunctionType
OP = mybir.AluOpType

D_MODEL = 512
D_INNER = 1024
D_STATE = 16
D_CONV = 4
DT_RANK = 32
BATCH = 8
SEQ = 1024

P = 128
NC_D = D_INNER // P   # 8 d-chunks
NC_T = SEQ // P       # 8 t-chunks
NN = SEQ // 512       # 2 psum-free blocks
NSP = D_STATE // 2    # 8 state pairs

# Engine split for the per-(c,sp) muls, tuned against the tile cost model:
# b2-mul always on Pool; hc2-mul on Pool every POOL_HC_MOD-th pair.
POOL_HC_MOD = 2


def _dir_params(nc, d):
    return {
        # packed layouts (see _prep_dir): single-DMA loads
        "inwT": nc.declare_dram_parameter(f"inwT_{d}", [P, 4 * 2 * D_INNER], BF16, isOutput=False),
        "dtwT": nc.declare_dram_parameter(f"dtwT_{d}", [DT_RANK, D_INNER], BF16, isOutput=False),
        "outwT": nc.declare_dram_parameter(f"outwT_{d}", [P, NC_D * D_MODEL], BF16, isOutput=False),
        "smallbf": nc.declare_dram_parameter(f"smallbf_{d}", [P, NC_D * 64], BF16, isOutput=False),
        "smallf32": nc.declare_dram_parameter(f"smallf32_{d}", [P, NC_D * (D_STATE + 2)], F32, isOutput=False),
        "convdiag": nc.declare_dram_parameter(f"convdiag_{d}", [P, NC_D * D_CONV * P], BF16, isOutput=False),
        "ddiag": nc.declare_dram_parameter(f"ddiag_{d}", [P, NC_D * P], BF16, isOutput=False),
        "out": nc.declare_dram_parameter(f"out_{d}", [SEQ, D_MODEL], F32, isOutput=True),
    }


class Shared:
    pass


def _build_shared(nc, tc, pools):
    """Pools + direction-independent tiles (identity, oht, x)."""
    sh = Shared()
    sh.xT_d = nc.declare_dram_parameter("xT", [D_MODEL, SEQ], BF16, isOutput=False)
    sh.oht_d = nc.declare_dram_parameter("oht", [2 * D_STATE, 2 * D_STATE * P], BF16, isOutput=False)
    sh.ident_d = nc.declare_dram_parameter("ident", [P, P], BF16, isOutput=False)

    cst = pools["cst"]
    sh.xT = [cst.tile([P, SEQ], BF16, tag=f"xT{k}", name=f"xT{k}") for k in range(4)]
    for k in range(4):
        nc.sync.dma_start(sh.xT[k][:], sh.xT_d[k * P:(k + 1) * P, :])
    sh.ident = cst.tile([P, P], BF16, tag="ident", name="ident")
    nc.sync.dma_start(sh.ident[:], sh.ident_d[:])
    return sh


def _one_direction(nc, tc, pools, sh, p, rev):
    """Emit one direction. rev=True reads x time-reversed."""
    cst = pools["cst"]
    wpool = pools["w"]          # rotating weight tiles (per-dir tags shared)
    ps_small = pools["ps_small"]
    ps_y = pools["ps_y"]

    # ---- weights (rotating by shared tags; dir b reuses dir f's buffers) ----
    inwT_t = wpool.tile([P, 4 * 2 * D_INNER], BF16, tag="inwT", name="inwT")
    nc.sync.dma_start(inwT_t[:], p["inwT"][:])
    inwT = [inwT_t[:, k * 2 * D_INNER:(k + 1) * 2 * D_INNER] for k in range(4)]
    smallbf = wpool.tile([P, NC_D * 64], BF16, tag="smallbf", name="smallbf")
    nc.sync.dma_start(smallbf[:], p["smallbf"][:])
    smallf32 = wpool.tile([P, NC_D * (D_STATE + 2)], F32, tag="smallf32", name="smallf32")
    nc.sync.dma_start(smallf32[:], p["smallf32"][:])
    xpwT = [smallbf[:, c * 64:(c + 1) * 64] for c in range(NC_D)]
    A_sb = [smallf32[:, c * (D_STATE + 2):c * (D_STATE + 2) + D_STATE] for c in range(NC_D)]
    convb = [smallf32[:, c * (D_STATE + 2) + D_STATE:c * (D_STATE + 2) + D_STATE + 1] for c in range(NC_D)]
    dtb = [smallf32[:, c * (D_STATE + 2) + D_STATE + 1:c * (D_STATE + 2) + D_STATE + 2] for c in range(NC_D)]
    dtwT = wpool.tile([DT_RANK, D_INNER], BF16, tag="dtwT", name="dtwT")
    nc.sync.dma_start(dtwT[:], p["dtwT"][:])

    uc = [pools["uc"].tile([P, SEQ], BF16, tag="uc", name=f"uc{c}") for c in range(NC_D)]
    sz = [pools["sz"].tile([P, SEQ], BF16, tag="sz", name=f"sz{c}") for c in range(NC_D)]

    def xs(n):
        """x k-tile slice for GEMM1 block n, reversed for the bwd dir."""
        if not rev:
            return [sh.xT[k][:, n * 512:(n + 1) * 512] for k in range(4)]
        # reversed block n reads original columns SEQ-1-n*512 .. SEQ-(n+1)*512
        start = SEQ - 1 - n * 512
        stop = SEQ - 1 - (n + 1) * 512
        if stop < 0:
            return [sh.xT[k][:, start::-1] for k in range(4)]
        return [sh.xT[k][:, start:stop:-1] for k in range(4)]

    # ---- GEMM1-u + conv fused per chunk ----
    for c in range(NC_D):
        uT = pools["uT"].tile([P, SEQ + D_CONV - 1], BF16, tag="uT", name=f"uT{c}")
        nc.vector.memset(uT[:, 0:D_CONV - 1], 0.0)
        for n in range(NN):
            pt = ps_small.tile([P, 512], F32, tag="g1", name="g1u")
            for k in range(4):
                nc.tensor.matmul(
                    pt[:], inwT[k][:, c * P:(c + 1) * P], xs(n)[k],
                    start=(k == 0), stop=(k == 3),
                )
            nc.scalar.copy(uT[:, D_CONV - 1 + n * 512:D_CONV - 1 + (n + 1) * 512], pt[:])
        # conv: uc_psum = sum_k diag(convw_k) @ u(t+k-3)
        cdt = pools["cdiag"].tile([P, D_CONV * P], BF16, tag="cdiag", name=f"cd{c}")
        nc.sync.dma_start(cdt[:], p["convdiag"][:, c * D_CONV * P:(c + 1) * D_CONV * P])
        cd = [cdt[:, k * P:(k + 1) * P] for k in range(D_CONV)]
        for n in range(NN):
            pt = ps_small.tile([P, 512], F32, tag="g1", name="g1c")
            for k in range(D_CONV):
                nc.tensor.matmul(
                    pt[:], cd[k], uT[:, k + n * 512:k + n * 512 + 512],
                    start=(k == 0), stop=(k == D_CONV - 1),
                )
            nc.scalar.activation(
                uc[c][:, n * 512:(n + 1) * 512], pt[:], AF.Silu, bias=convb[c]
            )

    # ---- GEMM1-z + silu ----
    for c in range(NC_D):
        for n in range(NN):
            pt = ps_small.tile([P, 512], F32, tag="g1", name="g1z")
            for k in range(4):
                nc.tensor.matmul(
                    pt[:], inwT[k][:, D_INNER + c * P:D_INNER + (c + 1) * P], xs(n)[k],
                    start=(k == 0), stop=(k == 3),
                )
            nc.scalar.activation(sz[c][:, n * 512:(n + 1) * 512], pt[:], AF.Silu)

    # ---- GEMM2: dbl = xp_w @ uc ----
    dt_bf = pools["dtbf"].tile([DT_RANK, SEQ], BF16, tag="dt_bf", name="dt_bf")
    bc_bf = pools["bcbf"].tile([2 * D_STATE, SEQ], BF16, tag="bc_bf", name="bc_bf")
    for n in range(NN):
        pt = ps_small.tile([64, 512], F32, tag="g1", name="g2")
        for c in range(NC_D):
            nc.tensor.matmul(
                pt[:], xpwT[c][:], uc[c][:, n * 512:(n + 1) * 512],
                start=(c == 0), stop=(c == NC_D - 1),
            )
        nc.vector.tensor_copy(dt_bf[:, n * 512:(n + 1) * 512], pt[0:DT_RANK, :])
        nc.vector.tensor_copy(bc_bf[:, n * 512:(n + 1) * 512], pt[DT_RANK:64, :])

    # ---- broadcast B/C rows to wide per-pair tiles [P, 2048] ----
    Bbc = [pools["bc"].tile([P, 2048], BF16, tag=f"Bbc{sp}", name=f"Bbc{sp}") for sp in range(NSP)]
    Cbc = [pools["bc"].tile([P, 2048], BF16, tag=f"Cbc{sp}", name=f"Cbc{sp}") for sp in range(NSP)]
    for sp in range(NSP):
        ohs = pools["oht"].tile([2 * D_STATE, 4 * P], BF16, tag="oht", name=f"oh{sp}")
        # [B_2sp | B_2sp+1 | C_2sp | C_2sp+1] one-hot column blocks
        nc.sync.dma_start(
            ohs[:].rearrange("r (h q) -> r h q", h=2),
            sh.oht_d[:].rearrange("r (h q) -> r h q", h=2)[:, :, sp * 2 * P:(sp * 2 + 2) * P],
        )
        cpy = 0
        for half, dst in ((0, Bbc[sp]), (1, Cbc[sp])):
            for j in range(2):
                for n in range(NN):
                    pt = ps_small.tile([P, 512], F32, tag="g1", name="bcb")
                    nc.tensor.matmul(
                        pt[:], ohs[:, (half * 2 + j) * P:(half * 2 + j + 1) * P],
                        bc_bf[:, n * 512:(n + 1) * 512],
                        start=True, stop=True,
                    )
                    if cpy % 2 == 0:
                        nc.scalar.copy(
                            dst[:, j * 1024 + n * 512:j * 1024 + (n + 1) * 512], pt[:]
                        )
                    else:
                        nc.vector.tensor_copy(
                            dst[:, j * 1024 + n * 512:j * 1024 + (n + 1) * 512], pt[:]
                        )
                    cpy += 1

    # ---- scan phase, chunk-outer ----
    ddt = pools["ddiag"].tile([P, NC_D * P], BF16, tag="ddiag", name="ddiag")
    nc.sync.dma_start(ddt[:], p["ddiag"][:])
    yg = [pools["yg"].tile([P, SEQ], BF16, tag="yg", name=f"yg{c}") for c in range(NC_D)]
    for c in range(NC_D):
        # lazy delta_c = softplus(dt_w @ dt + dtb), w_c = delta_c * uc_c
        pt = ps_small.tile([P, 512], F32, tag="g1", name="g3a")
        pt2 = ps_small.tile([P, 512], F32, tag="g1", name="g3b")
        nc.tensor.matmul(pt[:], dtwT[:, c * P:(c + 1) * P], dt_bf[:, 0:512],
                         start=True, stop=True)
        nc.tensor.matmul(pt2[:], dtwT[:, c * P:(c + 1) * P], dt_bf[:, 512:1024],
                         start=True, stop=True)
        delta = pools["delta"].tile([P, SEQ], BF16, tag="delta", name=f"delta{c}")
        nc.scalar.activation(delta[:, 0:512], pt[:], AF.Exp, bias=dtb[c])
        nc.scalar.activation(delta[:, 512:1024], pt2[:], AF.Exp, bias=dtb[c])
        nc.scalar.activation(delta[:], delta[:], AF.Ln, bias=1.0)
        w_t = pools["wt"].tile([P, SEQ], BF16, tag="wt", name=f"w{c}")
        nc.vector.tensor_mul(w_t[:], delta[:], uc[c][:])
        w_b = w_t[:].unsqueeze(1).broadcast_to((P, 2, 1024))

        # D * uc seeds the PSUM accumulator
        py = ps_y.tile([P, SEQ], F32, tag="py", name=f"py{c}")
        for n in range(NN):
            nc.tensor.matmul(py[:, n * 512:(n + 1) * 512], ddt[:, c * P:(c + 1) * P],
                             uc[c][:, n * 512:(n + 1) * 512], start=True, stop=False)

        for sp in range(NSP):
            a2 = pools["a2"].tile([P, 2048], BF16, tag="a2", name="a2")
            for j in range(2):
                nc.scalar.activation(
                    a2[:, j * 1024:(j + 1) * 1024], delta[:], AF.Exp,
                    scale=A_sb[c][:, sp * 2 + j:sp * 2 + j + 1],
                )
            b2 = pools["b2"].tile([P, 2048], BF16, tag="b2", name="b2")
            nc.gpsimd.tensor_tensor(
                b2[:].rearrange("p (s n) -> p s n", s=2), w_b,
                Bbc[sp][:].rearrange("p (s n) -> p s n", s=2), OP.mult,
            )
            h2 = pools["h2"].tile([P, 2048], BF16, tag="h2", name="h2")
            for j in range(2):
                nc.vector.tensor_tensor_scan(
                    h2[:, j * 1024:(j + 1) * 1024],
                    a2[:, j * 1024:(j + 1) * 1024],
                    b2[:, j * 1024:(j + 1) * 1024],
                    0.0, op0=OP.mult, op1=OP.add,
                )
            hc2 = pools["hc2"].tile([P, 2048], BF16, tag="hc2", name="hc2")
            hmul = nc.gpsimd if (c * NSP + sp) % POOL_HC_MOD == 0 else nc.vector
            hmul.tensor_mul(hc2[:], h2[:], Cbc[sp][:])
            last = sp == NSP - 1
            for j in range(2):
                for n in range(NN):
                    nc.tensor.matmul(
                        py[:, n * 512:(n + 1) * 512], sh.ident[:],
                        hc2[:, j * 1024 + n * 512:j * 1024 + (n + 1) * 512],
                        start=False, stop=(last and j == 1),
                    )
        # gate
        nc.vector.tensor_mul(yg[c][:], py[:], sz[c][:])

    # ---- GEMM4 ----
    outwT_t = wpool.tile([P, NC_D * D_MODEL], BF16, tag="outwT", name="outwT")
    nc.sync.dma_start(outwT_t[:], p["outwT"][:])
    outwT = [outwT_t[:, c * D_MODEL:(c + 1) * D_MODEL] for c in range(NC_D)]
    for m in range(NC_T):
        pt = ps_small.tile([P, D_MODEL], F32, tag="g1", name="g4")
        for c in range(NC_D):
            nc.tensor.matmul(
                pt[:], yg[c][:, m * P:(m + 1) * P], outwT[c],
                start=(c == 0), stop=(c == NC_D - 1),
            )
        ot = pools["g4o"].tile([P, D_MODEL], F32, tag="g4o", name="ot")
        nc.scalar.copy(ot[:], pt[:])
        nc.gpsimd.dma_start(p["out"][m * P:(m + 1) * P, :], ot[:])


def _split_excess_waits(nc):
    """walrus accepts at most one sync-wait per instruction (two for
    EventSemaphore); hoist the excess onto injected same-engine NoOps."""
    for f in nc.m.functions:
        for bb in f.blocks:
            new_insts = []
            for inst in bb.instructions:
                si = inst.sync_info
                cap = 2 if isinstance(inst, mybir.InstEventSemaphore) else 1
                if si is not None and len(si.on_wait) > cap:
                    waits = list(si.on_wait)
                    for i, w in enumerate(waits[:-cap]):
                        nop = mybir.InstNoOp(
                            name=f"{inst.name}-wsplit{i}", ins=[], outs=[]
                        )
                        nop.engine = inst.engine
                        nop.sync_info = bass_rust.SyncInfo(on_wait=[w], on_update=[])
                        new_insts.append(nop)
                    inst.sync_info = bass_rust.SyncInfo(
                        on_wait=waits[-cap:], on_update=list(si.on_update)
                    )
                new_insts.append(inst)
            try:
                bb.instructions = new_insts
            except Exception:
                bb.instructions.clear()
                bb.instructions.extend(new_insts)


def build_bass():
    nc = bass.Bass()
    params = {d: _dir_params(nc, d) for d in ("f", "b")}
    with tile.TileContext(nc) as tc:
        import contextlib
        with contextlib.ExitStack() as st:
            pools = {
                "cst": st.enter_context(tc.tile_pool(name="cst", bufs=1)),
                "w": st.enter_context(tc.tile_pool(name="w", bufs=1)),
                "uT": st.enter_context(tc.tile_pool(name="uT", bufs=2)),
                "uc": st.enter_context(tc.tile_pool(name="uc", bufs=8)),
                "sz": st.enter_context(tc.tile_pool(name="sz", bufs=8)),
                "yg": st.enter_context(tc.tile_pool(name="yg", bufs=8)),
                "delta": st.enter_context(tc.tile_pool(name="delta", bufs=2)),
                "wt": st.enter_context(tc.tile_pool(name="wt", bufs=2)),
                "dtbf": st.enter_context(tc.tile_pool(name="dtbf", bufs=2)),
                "bcbf": st.enter_context(tc.tile_pool(name="bcbf", bufs=1)),
                "bc": st.enter_context(tc.tile_pool(name="bc", bufs=1)),
                "cdiag": st.enter_context(tc.tile_pool(name="cdiag", bufs=2)),
                "ddiag": st.enter_context(tc.tile_pool(name="ddiag", bufs=1)),
                "oht": st.enter_context(tc.tile_pool(name="oht", bufs=2)),
                "a2": st.enter_context(tc.tile_pool(name="a2", bufs=2)),
                "b2": st.enter_context(tc.tile_pool(name="b2", bufs=2)),
                "h2": st.enter_context(tc.tile_pool(name="h2", bufs=2)),
                "hc2": st.enter_context(tc.tile_pool(name="hc2", bufs=2)),
                "g4o": st.enter_context(tc.tile_pool(name="g4o", bufs=1)),
                "ps_small": st.enter_context(tc.tile_pool(name="ps_small", bufs=4, space="PSUM")),
                "ps_y": st.enter_context(tc.tile_pool(name="ps_y", bufs=2, space="PSUM")),
            }
            sh = _build_shared(nc, tc, pools)
            _one_direction(nc, tc, pools, sh, params["f"], rev=False)
            _one_direction(nc, tc, pools, sh, params["b"], rev=True)
    _split_excess_waits(nc)
    return nc


def _prep_dir(w):
    bf = ml_dtypes.bfloat16
    in_w, conv_w, conv_b, xp_w, dt_w, dt_b, A_log, Dp, out_w = w
    in_wT = np.asarray(in_w, np.float32).T            # [512, 2048]
    out_wT = np.asarray(out_w, np.float32).T          # [1024, 512]
    xp_wT = np.asarray(xp_w, np.float32).T            # [1024, 64]
    conv_w = np.asarray(conv_w, np.float32)
    conv_b = np.asarray(conv_b, np.float32)
    dt_b = np.asarray(dt_b, np.float32)
    A = -np.exp(np.asarray(A_log, np.float64)).astype(np.float32)
    Dp = np.asarray(Dp, np.float32)

    # inwT packed [P, 4*2048]: k-block k holds in_wT rows k*128..k*128+127
    inwT = in_wT.reshape(4, P, 2 * D_INNER).transpose(1, 0, 2).reshape(P, 4 * 2 * D_INNER)
    # outwT packed [P, 8*512]: c-block holds out_wT rows c*128..
    outwT = out_wT.reshape(NC_D, P, D_MODEL).transpose(1, 0, 2).reshape(P, NC_D * D_MODEL)
    # smallbf [P, 8*64]: xp_wT rows per chunk
    smallbf = xp_wT.reshape(NC_D, P, 64).transpose(1, 0, 2).reshape(P, NC_D * 64)
    # smallf32 [P, 8*18]: per chunk [A(16) | conv_b | dt_b]
    sf = np.concatenate(
        [A.reshape(NC_D, P, D_STATE),
         conv_b.reshape(NC_D, P, 1),
         dt_b.reshape(NC_D, P, 1)], axis=2)
    smallf32 = sf.transpose(1, 0, 2).reshape(P, NC_D * (D_STATE + 2))
    # convdiag [P, 8*4*128]: block (c,k) = diag(conv_w[c*128: , k])
    convdiag = np.zeros((P, NC_D * D_CONV * P), np.float32)
    for c in range(NC_D):
        for k in range(D_CONV):
            blk = (c * D_CONV + k) * P
            convdiag[:, blk:blk + P] = np.diag(conv_w[c * P:(c + 1) * P, k])
    # ddiag [P, 8*128]
    ddiag = np.zeros((P, NC_D * P), np.float32)
    for c in range(NC_D):
        ddiag[:, c * P:(c + 1) * P] = np.diag(Dp[c * P:(c + 1) * P])
    return {
        "inwT": np.ascontiguousarray(inwT).astype(bf),
        "dtwT": np.ascontiguousarray(np.asarray(dt_w).T).astype(bf),
        "outwT": np.ascontiguousarray(outwT).astype(bf),
        "smallbf": np.ascontiguousarray(smallbf).astype(bf),
        "smallf32": np.ascontiguousarray(smallf32),
        "convdiag": np.ascontiguousarray(convdiag).astype(bf),
        "ddiag": np.ascontiguousarray(ddiag).astype(bf),
    }


_CACHED = {}


def kernel(
    x,
    in_w_f, conv_w_f, conv_b_f, xp_w_f, dt_w_f, dt_b_f, A_log_f, D_f, out_w_f,
    in_w_b, conv_w_b, conv_b_b, xp_w_b, dt_w_b, dt_b_b, A_log_b, D_b, out_w_b,
):
    bf = ml_dtypes.bfloat16
    x = np.asarray(x, dtype=np.float32)

    if "nc" not in _CACHED:
        _CACHED["nc"] = build_bass()
    nc = _CACHED["nc"]

    wf = _prep_dir((in_w_f, conv_w_f, conv_b_f, xp_w_f, dt_w_f, dt_b_f,
                    A_log_f, D_f, out_w_f))
    wb = _prep_dir((in_w_b, conv_w_b, conv_b_b, xp_w_b, dt_w_b, dt_b_b,
                    A_log_b, D_b, out_w_b))
    oht = np.kron(np.eye(2 * D_STATE, dtype=np.float32),
                  np.ones((1, P), np.float32)).astype(bf)
    ident = np.eye(P, dtype=np.float32).astype(bf)

    in_maps = []
    for b in range(BATCH):
        m = {"oht": oht, "ident": ident}
        for d, wd in (("f", wf), ("b", wb)):
            for k, v in wd.items():
                m[f"{k}_{d}"] = v
        m["xT"] = np.ascontiguousarray(x[b].T).astype(bf)
        in_maps.append(m)

    res = run_bass_kernel_spmd(nc, in_maps, core_ids=list(range(BATCH)))
    out = np.empty((BATCH, SEQ, D_MODEL), np.float32)
    for b in range(BATCH):
        rb = res.results[b]
        out[b] = rb["out_f"] + rb["out_b"][::-1]
    return out
